# revision 17
# baseline (speedup 1.0000x reference)
"""MLA (multi-head latent attention) Trainium2 Bass kernel — v3.

Sharding: 8 cores = batch(2) x head-groups(4 heads each); no collectives
(latents replicated per core); fc_out row-split with host-side partial sum.

v3 structure: software-pipelined segments — for each 512-wide seq chunk,
phase A (latents + q/k/v projections) is immediately followed by phase B
(attention q-block of the same index), so the Activation engine's exp work
overlaps A's matmul-only windows and the out-projection of block qb-1 is
interleaved as PE filler inside block qb's score/AV loop.

Attention uses transposed scores S_T[j, q] and a transposed AV
(av^T[d, q] = V^T ES, full-width streams, no PE transposes). The softmax
denominator is accumulated on DVE (zacc += es) and partition-reduced on
gpsimd, then folded into the PSUM->SBUF eviction of av^T.
RMSNorm stats run on gpsimd (partition reduce) + one Act Rsqrt.
"""

import sys
import numpy as np
import ml_dtypes

for _p in ("/opt/trn_rl_repo", "/root/.axon_site/_ro/trn_rl_repo"):
    if _p not in sys.path:
        sys.path.append(_p)

BF16 = ml_dtypes.bfloat16

D_MODEL = 2048
SEQ = 2048
BATCH = 2
N_HEADS = 16
D_HEAD = 128
D_KV = 512
D_ROPE = 64
ROPE_BASE = 10000.0
EPS = 1e-5
H_LOC = 4          # heads per core
N_CORES = 8

_BUILD_CACHE = {}


def build_program(reps: int = 1):
    if reps in _BUILD_CACHE:
        return _BUILD_CACHE[reps]

    import concourse.bass as bass  # noqa: F401
    import concourse.mybir as mybir
    from concourse import bacc, bass_isa
    from concourse.tile import TileContext
    from contextlib import ExitStack

    f32 = mybir.dt.float32
    bf16 = mybir.dt.bfloat16
    AF = mybir.ActivationFunctionType
    OP = mybir.AluOpType
    AX = mybir.AxisListType

    nc = bacc.Bacc(num_devices=8)

    xT = nc.declare_dram_parameter("xT", [D_MODEL, SEQ], bf16, isOutput=False)
    wdq = nc.declare_dram_parameter("wdq", [D_MODEL, D_KV], bf16, isOutput=False)
    wdkv = nc.declare_dram_parameter("wdkv", [D_MODEL, D_KV], bf16, isOutput=False)
    wq = nc.declare_dram_parameter("wq", [D_KV, H_LOC * 128], bf16, isOutput=False)
    wuk2 = nc.declare_dram_parameter("wuk2", [D_KV, 2 * 128], bf16, isOutput=False)
    wkr2 = nc.declare_dram_parameter("wkr2", [D_MODEL, 2 * 128], bf16, isOutput=False)
    wuv = nc.declare_dram_parameter("wuv", [D_KV, H_LOC * 128], bf16, isOutput=False)
    wout = nc.declare_dram_parameter("wout", [H_LOC * 128, D_MODEL], bf16, isOutput=False)
    mult = nc.declare_dram_parameter("mult", [128, 2 * SEQ], bf16, isOutput=False)
    masks = nc.declare_dram_parameter("masks", [128, 896], bf16, isOutput=False)
    y = nc.declare_dram_parameter("y", [SEQ, D_MODEL], bf16, isOutput=True)

    SCALE = 1.0 / float(np.sqrt(np.float32(D_HEAD)))
    NKT = D_MODEL // 128    # 16 k-tiles over d_model
    NLT = D_KV // 128       # 4  k-tiles over latent
    NSN = SEQ // 512        # 4  sequence chunks / q blocks

    with TileContext(nc) as tc, ExitStack() as top:
        pp = top.enter_context(tc.tile_pool(name="persist", bufs=1))
        qt_sb = pp.tile([128, H_LOC * SEQ], bf16, tag="qt")
        kt_sb = pp.tile([128, H_LOC * SEQ], bf16, tag="kt")
        v_sb = pp.tile([128, (SEQ // 128) * H_LOC * 128], bf16, tag="v")

        pW = top.enter_context(tc.tile_pool(name="pW", bufs=1))
        pX = top.enter_context(tc.tile_pool(name="pX", bufs=3))
        pCp = top.enter_context(tc.tile_pool(name="pCp", bufs=6))
        pSq = top.enter_context(tc.tile_pool(name="pSq", bufs=2))
        pCn = top.enter_context(tc.tile_pool(name="pCn", bufs=2))
        pBc = top.enter_context(tc.tile_pool(name="pBc", bufs=3))
        pEs = top.enter_context(tc.tile_pool(name="pEs", bufs=5))
        pZa = top.enter_context(tc.tile_pool(name="pZa", bufs=3))
        pAv = top.enter_context(tc.tile_pool(name="pAv", bufs=8))
        pYs = top.enter_context(tc.tile_pool(name="pYs", bufs=4))
        psBig = top.enter_context(tc.tile_pool(name="psBig", bufs=2, space="PSUM"))
        psAv = top.enter_context(tc.tile_pool(name="psAv", bufs=2, space="PSUM"))
        psYp = top.enter_context(tc.tile_pool(name="psYp", bufs=2, space="PSUM"))

        wdq_sb = pW.tile([128, NKT * D_KV], bf16, tag="wdq")
        wdkv_sb = pW.tile([128, NKT * D_KV], bf16, tag="wdkv")
        wq_sb = pW.tile([128, NLT * 512], bf16, tag="wq")
        wuk2_sb = pW.tile([128, NLT * 256], bf16, tag="wuk2")
        wkr2_sb = pW.tile([128, NKT * 256], bf16, tag="wkr2")
        wuv_sb = pW.tile([128, NLT * 512], bf16, tag="wuv")
        wout_sb = pW.tile([128, NLT * D_MODEL], bf16, tag="wout")
        mult_sb = pW.tile([128, 2 * SEQ], bf16, tag="mult")
        masks_sb = pW.tile([128, 896], bf16, tag="masks")
        epsb = pW.tile([128, 1], f32, tag="epsb")
        nc.vector.memset(epsb[:], EPS)

        def mask_sl(kd, lo, hi):
            off = 384 - 128 * kd
            return masks_sb[:, off + lo: off + hi]

        for kt in range(NKT):
            nc.sync.dma_start(out=wdq_sb[:, kt * D_KV:(kt + 1) * D_KV],
                              in_=wdq[kt * 128:(kt + 1) * 128, :])
            nc.gpsimd.dma_start(out=wdkv_sb[:, kt * D_KV:(kt + 1) * D_KV],
                                in_=wdkv[kt * 128:(kt + 1) * 128, :])
        for kt in range(NKT):
            nc.gpsimd.dma_start(out=wkr2_sb[:, kt * 256:(kt + 1) * 256],
                                in_=wkr2[kt * 128:(kt + 1) * 128, :])
        nc.sync.dma_start(out=mult_sb[:], in_=mult[:, :])
        for lt in range(NLT):
            nc.sync.dma_start(out=wq_sb[:, lt * 512:(lt + 1) * 512],
                              in_=wq[lt * 128:(lt + 1) * 128, :])
            nc.gpsimd.dma_start(out=wuk2_sb[:, lt * 256:(lt + 1) * 256],
                                in_=wuk2[lt * 128:(lt + 1) * 128, :])
            nc.sync.dma_start(out=wuv_sb[:, lt * 512:(lt + 1) * 512],
                              in_=wuv[lt * 128:(lt + 1) * 128, :])
        nc.gpsimd.dma_start(out=masks_sb[:], in_=masks[:, :])
        for lt in range(NLT):
            nc.gpsimd.dma_start(out=wout_sb[:, lt * D_MODEL:(lt + 1) * D_MODEL],
                                in_=wout[lt * 128:(lt + 1) * 128, :])

        for _rep in range(reps):
            pending = []       # out-projection thunks from the previous q-block

            def emit_pending(budget):
                for _ in range(min(budget, len(pending))):
                    pending.pop(0)()

            def dma_xts_half(sn, hx):
                xt_t = pX.tile([128, 8 * 512], bf16, name=f"xts{sn}_{hx}", tag="xts")
                for k8 in range(8):
                    kt = hx * 8 + k8
                    dq = nc.sync if k8 % 2 == 0 else nc.gpsimd
                    dq.dma_start(
                        out=xt_t[:, k8 * 512:(k8 + 1) * 512],
                        in_=xT[kt * 128:(kt + 1) * 128, sn * 512:(sn + 1) * 512])
                return xt_t

            prefetched = dma_xts_half(0, 0)
            for seg in range(NSN):
                sn = seg
                # ================= A(sn): latents + projections =================
                xth = [prefetched, dma_xts_half(sn, 1)]

                def xts_t(kt):
                    return xth[kt // 8][:, (kt % 8) * 512:(kt % 8 + 1) * 512]

                cps_all = {}
                for cname, wd_sb in (("q", wdq_sb), ("kv", wdkv_sb)):
                    for ltp in range(2):
                        cp2 = psBig.tile([128, 1024], f32, tag="mm",
                                         name=f"cp2{cname}{sn}{ltp}")
                        for half in range(2):
                            lt = 2 * ltp + half
                            for kt in range(NKT):
                                nc.tensor.matmul(
                                    cp2[:, half * 512:(half + 1) * 512],
                                    wd_sb[:, kt * D_KV + lt * 128: kt * D_KV + (lt + 1) * 128],
                                    xts_t(kt),
                                    start=(kt == 0), stop=(kt == NKT - 1))
                            cps = pCp.tile([128, 512], bf16, tag="cpre",
                                           name=f"cpre{cname}{sn}{lt}")
                            nc.scalar.copy(cps[:], cp2[:, half * 512:(half + 1) * 512])
                            cps_all[(cname, lt)] = cps

                # kr chains for this sn (keeps PE busy while stats settle)
                kp2 = psBig.tile([128, 1024], f32, tag="mm", name=f"kr2{sn}")
                for p in range(2):
                    for kt in range(NKT):
                        nc.tensor.matmul(
                            kp2[:, p * 512:(p + 1) * 512],
                            wkr2_sb[:, kt * 256 + p * 128: kt * 256 + (p + 1) * 128],
                            xts_t(kt),
                            start=(kt == 0), stop=(kt == NKT - 1))

                # rmsnorm stats: squares on Act, adds on DVE, one gpsimd
                # partition_all_reduce per latent, then
                # bstd = exp(-0.5*ln(ss/512+eps)) on Act (same table set as
                # the attention exp -> no act-table swaps)
                cn = {}
                for cname in ("q", "kv"):
                    sqs = []
                    for lt in range(NLT):
                        sq = pSq.tile([128, 512], bf16, tag="sq",
                                      name=f"sq{cname}{sn}{lt}")
                        nc.scalar.activation(sq[:], cps_all[(cname, lt)][:],
                                             AF.Square)
                        sqs.append(sq)
                    s01 = pSq.tile([128, 512], f32, tag="sqs", bufs=3, name=f"s01{cname}{sn}")
                    s23 = pSq.tile([128, 512], f32, tag="sqs", bufs=3, name=f"s23{cname}{sn}")
                    stot = pSq.tile([128, 512], f32, tag="sqs", bufs=3, name=f"stot{cname}{sn}")
                    nc.vector.tensor_tensor(s01[:], sqs[0][:], sqs[1][:], OP.add)
                    nc.vector.tensor_tensor(s23[:], sqs[2][:], sqs[3][:], OP.add)
                    nc.vector.tensor_tensor(stot[:], s01[:], s23[:], OP.add)
                    sall = pBc.tile([128, 512], f32, tag="tmp", name=f"sall{cname}{sn}")
                    nc.gpsimd.partition_all_reduce(sall[:], stot[:], 128,
                                                   bass_isa.ReduceOp.add)
                    lnv = pBc.tile([128, 512], f32, tag="tmp", name=f"lnv{cname}{sn}")
                    nc.scalar.activation(lnv[:], sall[:], AF.Ln,
                                         bias=epsb[:], scale=1.0 / D_KV)
                    bstd = pBc.tile([128, 512], f32, tag="tmp",
                                    name=f"bstd{cname}{sn}")
                    nc.scalar.activation(bstd[:], lnv[:], AF.Exp, scale=-0.5)
                    cfull = pCn.tile([128, NLT * 512], bf16, tag="cn",
                                     name=f"cn{cname}{sn}")
                    for lt in range(NLT):
                        nc.vector.tensor_tensor(
                            cfull[:, lt * 512:(lt + 1) * 512],
                            cps_all[(cname, lt)][:], bstd[:], OP.mult)
                    cn[cname] = cfull

                if sn + 1 < NSN:
                    prefetched = dma_xts_half(sn + 1, 0)

                # kr rope-multiply into kt_sb
                for p in range(2):
                    h0, h1 = 2 * p, 2 * p + 1
                    kp = kp2[:, p * 512:(p + 1) * 512]
                    m0 = mult_sb[:, 0 * SEQ + sn * 512: 0 * SEQ + (sn + 1) * 512]
                    m1 = mult_sb[:, 1 * SEQ + sn * 512: 1 * SEQ + (sn + 1) * 512]
                    k0 = kt_sb[:, h0 * SEQ + sn * 512: h0 * SEQ + (sn + 1) * 512]
                    k1 = kt_sb[:, h1 * SEQ + sn * 512: h1 * SEQ + (sn + 1) * 512]
                    nc.vector.tensor_tensor(k0[64:128, :], kp[64:128, :], m0[64:128, :], OP.mult)
                    nc.vector.tensor_tensor(k1[0:64, :], kp[0:64, :], m1[0:64, :], OP.mult)

                def cnq(lt):
                    return cn["q"][:, lt * 512:(lt + 1) * 512]

                def cnkv(lt):
                    return cn["kv"][:, lt * 512:(lt + 1) * 512]

                # q projections (pairs of heads share one PSUM pair-tile)
                for hp in range(2):
                    qp2 = psBig.tile([128, 1024], f32, tag="mm", name=f"qp2{sn}{hp}")
                    for half in range(2):
                        hl = 2 * hp + half
                        for lt in range(NLT):
                            nc.tensor.matmul(
                                qp2[:, half * 512:(half + 1) * 512],
                                wq_sb[:, lt * 512 + hl * 128: lt * 512 + (hl + 1) * 128],
                                cnq(lt),
                                start=(lt == 0), stop=(lt == NLT - 1))
                    for half in range(2):
                        hl = 2 * hp + half
                        nc.vector.tensor_tensor(
                            qt_sb[:, hl * SEQ + sn * 512: hl * SEQ + (sn + 1) * 512],
                            qp2[:, half * 512:(half + 1) * 512],
                            mult_sb[:, (hl % 2) * SEQ + sn * 512: (hl % 2) * SEQ + (sn + 1) * 512],
                            OP.mult)
                # k-base projections
                up2 = psBig.tile([128, 1024], f32, tag="mm", name=f"up2{sn}")
                for p in range(2):
                    for lt in range(NLT):
                        nc.tensor.matmul(
                            up2[:, p * 512:(p + 1) * 512],
                            wuk2_sb[:, lt * 256 + p * 128: lt * 256 + (p + 1) * 128],
                            cnkv(lt),
                            start=(lt == 0), stop=(lt == NLT - 1))
                for p in range(2):
                    h0, h1 = 2 * p, 2 * p + 1
                    up = up2[:, p * 512:(p + 1) * 512]
                    m0 = mult_sb[:, 0 * SEQ + sn * 512: 0 * SEQ + (sn + 1) * 512]
                    m1 = mult_sb[:, 1 * SEQ + sn * 512: 1 * SEQ + (sn + 1) * 512]
                    k0 = kt_sb[:, h0 * SEQ + sn * 512: h0 * SEQ + (sn + 1) * 512]
                    k1 = kt_sb[:, h1 * SEQ + sn * 512: h1 * SEQ + (sn + 1) * 512]
                    nc.vector.tensor_tensor(k0[0:64, :], up[0:64, :], m0[0:64, :], OP.mult)
                    nc.vector.tensor_tensor(k1[64:128, :], up[64:128, :], m1[64:128, :], OP.mult)
                # v projections: [seq part, (h d) free], pairs of 128-row s-tiles
                for stp in range(2):
                    vp2 = psBig.tile([128, 1024], f32, tag="mm", name=f"vp2{sn}{stp}")
                    for half in range(2):
                        st = 2 * stp + half
                        for lt in range(NLT):
                            nc.tensor.matmul(
                                vp2[:, half * 512:(half + 1) * 512],
                                cnkv(lt)[:, st * 128:(st + 1) * 128],
                                wuv_sb[:, lt * 512:(lt + 1) * 512],
                                start=(lt == 0), stop=(lt == NLT - 1))
                    s_tile0 = sn * 4 + 2 * stp
                    nc.scalar.copy(
                        v_sb[:, s_tile0 * 512:(s_tile0 + 2) * 512], vp2[:])

                # ================= B(qb=seg): attention q-block =================
                qb = seg
                npair = (qb + 1) * 2
                av_sb = {}
                # budget out-projection filler: spread previous block's 16 chains
                nslots = npair * 2
                per_slot = -(-len(pending) // nslots) if pending else 0
                for hp in range(2):
                    heads = (2 * hp, 2 * hp + 1)
                    av = {}
                    for h in heads:
                        av[h] = psAv.tile([128, 512], f32, tag="av",
                                          name=f"av{qb}{h}")
                    # per-partition softmax partial sums for both heads of the
                    # pair, bf16 (2x DVE), reduced by ONE gpsimd call
                    za2 = pZa.tile([128, 1024], bf16, tag="za",
                                   name=f"za2{qb}{hp}")

                    def zsl(h, lo, hi):
                        zoff = (h % 2) * 512
                        return za2[:, zoff + lo: zoff + hi]
                    es2s = {}
                    for jp in range(npair):
                        jt0, jt1 = 2 * jp, 2 * jp + 1
                        kd0, kd1 = jt0 - qb * 4, jt1 - qb * 4
                        c0e = max(kd0, 0) * 128
                        c0o = max(kd1, 0) * 128
                        for h in heads:
                            sp2 = psBig.tile([128, 1024], f32, tag="mm",
                                             name=f"sp{qb}{h}{jp}")
                            nc.tensor.matmul(
                                sp2[:, c0e:512],
                                kt_sb[:, h * SEQ + jt0 * 128: h * SEQ + (jt0 + 1) * 128],
                                qt_sb[:, h * SEQ + qb * 512 + c0e: h * SEQ + (qb + 1) * 512],
                                start=True, stop=True)
                            # diagonal pairs write the full second half so the
                            # paired exp never reads unwritten PSUM
                            c0w = 0 if kd1 >= 0 else c0o
                            nc.tensor.matmul(
                                sp2[:, 512 + c0w:1024],
                                kt_sb[:, h * SEQ + jt1 * 128: h * SEQ + (jt1 + 1) * 128],
                                qt_sb[:, h * SEQ + qb * 512 + c0w: h * SEQ + (qb + 1) * 512],
                                start=True, stop=True)
                            es2s[h] = (sp2, jp)
                        emit_pending(per_slot)
                        for h in heads:
                            sp2, _ = es2s[h]
                            es2 = pEs.tile([128, 1024], bf16, tag="es",
                                           name=f"es{qb}{h}{jp}")
                            nc.scalar.activation(es2[:, c0e:], sp2[:, c0e:],
                                                 AF.Exp, scale=SCALE)
                            if kd0 >= 0:
                                if c0e > 0:
                                    nc.vector.memset(es2[:, 0:c0e], 0.0)
                                nc.vector.tensor_tensor(
                                    es2[:, c0e:512], es2[:, c0e:512],
                                    mask_sl(kd0, c0e, 512), OP.mult)
                                nc.vector.tensor_tensor(
                                    es2[:, 512:1024], es2[:, 512:1024],
                                    mask_sl(kd1, 0, 512), OP.mult)
                            if jp == 0:
                                nc.vector.tensor_copy(zsl(h, 0, 512), es2[:, 0:512])
                            else:
                                nc.vector.tensor_tensor(zsl(h, c0e, 512), zsl(h, c0e, 512),
                                                        es2[:, c0e:512], OP.add)
                            nc.vector.tensor_tensor(zsl(h, c0e, 512), zsl(h, c0e, 512),
                                                    es2[:, 512 + c0e:1024], OP.add)
                            for half, jt in ((0, jt0), (1, jt1)):
                                vsl = v_sb[:, (jt * H_LOC + h) * 128:
                                           (jt * H_LOC + h + 1) * 128]
                                nc.tensor.matmul(
                                    av[h][:],
                                    vsl,
                                    es2[:, half * 512:(half + 1) * 512],
                                    start=(jp == 0 and half == 0),
                                    stop=(jp == npair - 1 and half == 1))
                    zall2 = pBc.tile([128, 1024], bf16, tag="z2", bufs=2,
                                     name=f"zall2{qb}{hp}")
                    nc.gpsimd.partition_all_reduce(zall2[:], za2[:], 128,
                                                   bass_isa.ReduceOp.add)
                    rzb2 = pBc.tile([128, 1024], bf16, tag="z2", bufs=2,
                                    name=f"rzb2{qb}{hp}")
                    with nc.allow_low_precision(reason="1/z at bf16 is within budget"):
                        nc.vector.reciprocal(rzb2[:], zall2[:])
                    for h in heads:
                        zoff = (h % 2) * 512
                        avs = pAv.tile([128, 512], bf16, tag="avs", name=f"avs{qb}{h}")
                        nc.vector.tensor_tensor(avs[:], av[h][:],
                                                rzb2[:, zoff:zoff + 512], OP.mult)
                        av_sb[h] = avs

                # flush any remaining previous-block out-projection chains
                emit_pending(len(pending))

                # queue this block's out-projection (consumed during block qb+1)
                def mk_thunk(qb, av_tiles, st, ncol, parity):
                    def thunk():
                        yp = psYp.tile([128, 512], f32, tag="yp",
                                       name=f"yp{qb}{st}{ncol}")
                        for f in range(H_LOC):
                            nc.tensor.matmul(
                                yp[:],
                                av_tiles[f][:, st * 128:(st + 1) * 128],
                                wout_sb[:, f * D_MODEL + ncol * 512: f * D_MODEL + (ncol + 1) * 512],
                                start=(f == 0), stop=(f == H_LOC - 1))
                        ys = pYs.tile([128, 512], bf16, tag="ysb",
                                      name=f"ys{qb}{st}{ncol}")
                        nc.vector.tensor_copy(ys[:], yp[:])
                        row0 = qb * 512 + st * 128
                        dq = nc.sync if parity == 0 else nc.gpsimd
                        dq.dma_start(out=y[row0:row0 + 128, ncol * 512:(ncol + 1) * 512],
                                     in_=ys[:])
                    return thunk

                for st in range(4):
                    for ncol in range(4):
                        pending.append(mk_thunk(qb, av_sb, st, ncol, (st + ncol) % 2))

            # final block's out-projection
            emit_pending(len(pending))

    nc.finalize()
    _BUILD_CACHE[reps] = nc
    return nc


def _rope_mult():
    half = D_ROPE // 2
    theta = 1.0 / (ROPE_BASE ** (np.arange(0, D_HEAD, 2, dtype=np.float32) / D_HEAD))
    idx = np.arange(SEQ, dtype=np.float32)[:, None] * theta[None, :]
    r = np.tile(np.cos(idx[:, :half]), (1, 2)) + np.tile(np.sin(idx[:, :half]), (1, 2))
    return np.ascontiguousarray(r.T).astype(np.float32)  # [64, SEQ]


def make_inputs(x, W_dq, W_uq, W_dkv, W_uk, W_uv, W_qr, W_kr, g_q, g_kv, W_out, b_out):
    rT = _rope_mult()
    mult = np.empty((128, 2 * SEQ), np.float32)
    mult[0:64, 0:SEQ] = 1.0
    mult[64:128, 0:SEQ] = rT
    mult[0:64, SEQ:] = rT
    mult[64:128, SEQ:] = 1.0
    mult = mult.astype(BF16)

    jl = np.arange(128)[:, None]
    uu = np.arange(896)[None, :]
    masks = (uu >= 384 + jl).astype(np.float32).astype(BF16)

    gq = g_q.astype(np.float32)[:, None]
    gkv = g_kv.astype(np.float32)[:, None]
    Wuq_g = W_uq * gq
    Wqr_g = W_qr * gq
    Wuk_g = W_uk * gkv
    Wuv_g = W_uv * gkv

    in_maps = []
    for core in range(N_CORES):
        b = core // 4
        g = core % 4
        heads = [4 * g + i for i in range(H_LOC)]

        xb = np.ascontiguousarray(x[b].T).astype(BF16)  # [d_model, seq]

        wq_pack = np.empty((D_KV, H_LOC * 128), np.float32)
        for hl, h in enumerate(heads):
            a = Wuq_g[:, h * 64:(h + 1) * 64]
            r = Wqr_g[:, h * 64:(h + 1) * 64]
            blk = np.concatenate([a, r], axis=1) if hl % 2 == 0 else np.concatenate([r, a], axis=1)
            wq_pack[:, hl * 128:(hl + 1) * 128] = blk

        wuk2 = np.empty((D_KV, 256), np.float32)
        wkr2 = np.empty((D_MODEL, 256), np.float32)
        for p in range(2):
            h0, h1 = heads[2 * p], heads[2 * p + 1]
            wuk2[:, p * 128: p * 128 + 64] = Wuk_g[:, h0 * 64:(h0 + 1) * 64]
            wuk2[:, p * 128 + 64: p * 128 + 128] = Wuk_g[:, h1 * 64:(h1 + 1) * 64]
            wkr2[:, p * 128: p * 128 + 64] = W_kr[:, h1 * 64:(h1 + 1) * 64]
            wkr2[:, p * 128 + 64: p * 128 + 128] = W_kr[:, h0 * 64:(h0 + 1) * 64]

        wuv_pack = np.concatenate(
            [Wuv_g[:, h * 128:(h + 1) * 128] for h in heads], axis=1)
        wout_pack = np.concatenate(
            [W_out[h * 128:(h + 1) * 128, :] for h in heads], axis=0)

        in_maps.append({
            "xT": xb,
            "wdq": W_dq.astype(BF16),
            "wdkv": W_dkv.astype(BF16),
            "wq": wq_pack.astype(BF16),
            "wuk2": wuk2.astype(BF16),
            "wkr2": wkr2.astype(BF16),
            "wuv": wuv_pack.astype(BF16),
            "wout": wout_pack.astype(BF16),
            "mult": mult,
            "masks": masks,
        })
    return in_maps


def kernel(**inputs):
    inputs = {k: np.asarray(v) for k, v in inputs.items()}
    in_maps = make_inputs(
        inputs["x"], inputs["W_dq"], inputs["W_uq"], inputs["W_dkv"],
        inputs["W_uk"], inputs["W_uv"], inputs["W_qr"], inputs["W_kr"],
        inputs["g_q"], inputs["g_kv"], inputs["W_out"], inputs["b_out"])

    nc = build_program(reps=1)
    from concourse.bass_utils import run_bass_kernel_spmd
    res = run_bass_kernel_spmd(nc, in_maps, list(range(N_CORES)))

    b_out = inputs["b_out"].astype(np.float32)
    out = np.zeros((BATCH, SEQ, D_MODEL), np.float32)
    for core in range(N_CORES):
        out[core // 4] += res.results[core]["y"].astype(np.float32)
    out += b_out[None, None, :]
    return out



# revision 23
# speedup vs baseline: 1.1176x; 1.1176x over previous
"""MLA (multi-head latent attention) Trainium2 Bass kernel — v6b.

Sharding: 8 cores = batch(2) x head-groups(4 heads each); no collectives
(latents replicated per core); fc_out row-split with host-side partial sum.

Structure: software-pipelined segments — for each 512-wide seq chunk,
phase A (latents + q/k/v projections) is immediately followed by phase B
(attention q-block of the same index), so the Activation engine's exp work
overlaps A's matmul-only windows and the out-projection of block qb-1 is
interleaved as PE filler inside block qb's score/AV loop.

Attention uses transposed scores S_T[j, q] and a transposed AV
(av^T[d, q] = V^T ES, full-width streams, no PE transposes).

Engine balance (measured per-rep device time 472us (v3) -> ~390us):
- Phase-A PSUM evictions (latent cps, v projections) and the rmsnorm
  squares run on the otherwise-idle Act engine; rsqrt is DVE reciprocal
  feeding Act Ln/Exp (bstd = exp(-0.5 ln(ss/512+eps))).
- Softmax z: per-partition partial sums in bf16 on DVE (2x mode), both
  heads of a pair packed in one [128,1024] tile -> ONE gpsimd
  partition_all_reduce per pair (8/rep instead of 16), bf16 reciprocal.
- ALL xts / y DMA triggers on the SP (sync) queue — measured faster than
  splitting across SP/gpsimd, and never on Act (keeps phase-B Act
  exp-only). ys evictions on DVE. Masks stay on DVE (gpsimd tensor ops
  measured ~140us/rep slower).
"""

import sys
import numpy as np
import ml_dtypes

for _p in ("/opt/trn_rl_repo", "/root/.axon_site/_ro/trn_rl_repo"):
    if _p not in sys.path:
        sys.path.append(_p)

BF16 = ml_dtypes.bfloat16

D_MODEL = 2048
SEQ = 2048
BATCH = 2
N_HEADS = 16
D_HEAD = 128
D_KV = 512
D_ROPE = 64
ROPE_BASE = 10000.0
EPS = 1e-5
H_LOC = 4          # heads per core
N_CORES = 8

_BUILD_CACHE = {}


def build_program(reps: int = 1):
    if reps in _BUILD_CACHE:
        return _BUILD_CACHE[reps]

    import concourse.bass as bass  # noqa: F401
    import concourse.mybir as mybir
    from concourse import bacc, bass_isa
    from concourse.tile import TileContext
    from contextlib import ExitStack

    f32 = mybir.dt.float32
    bf16 = mybir.dt.bfloat16
    AF = mybir.ActivationFunctionType
    OP = mybir.AluOpType
    AX = mybir.AxisListType

    nc = bacc.Bacc(num_devices=8)

    xT = nc.declare_dram_parameter("xT", [D_MODEL, SEQ], bf16, isOutput=False)
    wdq = nc.declare_dram_parameter("wdq", [D_MODEL, D_KV], bf16, isOutput=False)
    wdkv = nc.declare_dram_parameter("wdkv", [D_MODEL, D_KV], bf16, isOutput=False)
    wq = nc.declare_dram_parameter("wq", [D_KV, H_LOC * 128], bf16, isOutput=False)
    wuk2 = nc.declare_dram_parameter("wuk2", [D_KV, 2 * 128], bf16, isOutput=False)
    wkr2 = nc.declare_dram_parameter("wkr2", [D_MODEL, 2 * 128], bf16, isOutput=False)
    wuv = nc.declare_dram_parameter("wuv", [D_KV, H_LOC * 128], bf16, isOutput=False)
    wout = nc.declare_dram_parameter("wout", [H_LOC * 128, D_MODEL], bf16, isOutput=False)
    mult = nc.declare_dram_parameter("mult", [128, 2 * SEQ], bf16, isOutput=False)
    masks = nc.declare_dram_parameter("masks", [128, 896], bf16, isOutput=False)
    y = nc.declare_dram_parameter("y", [SEQ, D_MODEL], bf16, isOutput=True)

    SCALE = 1.0 / float(np.sqrt(np.float32(D_HEAD)))
    NKT = D_MODEL // 128    # 16 k-tiles over d_model
    NLT = D_KV // 128       # 4  k-tiles over latent
    NSN = SEQ // 512        # 4  sequence chunks / q blocks

    with TileContext(nc) as tc, ExitStack() as top:
        pp = top.enter_context(tc.tile_pool(name="persist", bufs=1))
        qt_sb = pp.tile([128, H_LOC * SEQ], bf16, tag="qt")
        kt_sb = pp.tile([128, H_LOC * SEQ], bf16, tag="kt")
        v_sb = pp.tile([128, (SEQ // 128) * H_LOC * 128], bf16, tag="v")

        pW = top.enter_context(tc.tile_pool(name="pW", bufs=1))
        pX = top.enter_context(tc.tile_pool(name="pX", bufs=3))
        pCp = top.enter_context(tc.tile_pool(name="pCp", bufs=6))
        pSq = top.enter_context(tc.tile_pool(name="pSq", bufs=2))
        pCn = top.enter_context(tc.tile_pool(name="pCn", bufs=2))
        pBc = top.enter_context(tc.tile_pool(name="pBc", bufs=3))
        pEs = top.enter_context(tc.tile_pool(name="pEs", bufs=5))
        pZa = top.enter_context(tc.tile_pool(name="pZa", bufs=3))
        pAv = top.enter_context(tc.tile_pool(name="pAv", bufs=8))
        pYs = top.enter_context(tc.tile_pool(name="pYs", bufs=4))
        psBig = top.enter_context(tc.tile_pool(name="psBig", bufs=2, space="PSUM"))
        psAv = top.enter_context(tc.tile_pool(name="psAv", bufs=2, space="PSUM"))
        psYp = top.enter_context(tc.tile_pool(name="psYp", bufs=2, space="PSUM"))

        wdq_sb = pW.tile([128, NKT * D_KV], bf16, tag="wdq")
        wdkv_sb = pW.tile([128, NKT * D_KV], bf16, tag="wdkv")
        wq_sb = pW.tile([128, NLT * 512], bf16, tag="wq")
        wuk2_sb = pW.tile([128, NLT * 256], bf16, tag="wuk2")
        wkr2_sb = pW.tile([128, NKT * 256], bf16, tag="wkr2")
        wuv_sb = pW.tile([128, NLT * 512], bf16, tag="wuv")
        wout_sb = pW.tile([128, NLT * D_MODEL], bf16, tag="wout")
        mult_sb = pW.tile([128, 2 * SEQ], bf16, tag="mult")
        masks_sb = pW.tile([128, 896], bf16, tag="masks")
        epsb = pW.tile([128, 1], f32, tag="epsb")
        nc.vector.memset(epsb[:], EPS)

        def mask_sl(kd, lo, hi):
            off = 384 - 128 * kd
            return masks_sb[:, off + lo: off + hi]

        for kt in range(NKT):
            nc.sync.dma_start(out=wdq_sb[:, kt * D_KV:(kt + 1) * D_KV],
                              in_=wdq[kt * 128:(kt + 1) * 128, :])
            nc.gpsimd.dma_start(out=wdkv_sb[:, kt * D_KV:(kt + 1) * D_KV],
                                in_=wdkv[kt * 128:(kt + 1) * 128, :])
        for kt in range(NKT):
            nc.gpsimd.dma_start(out=wkr2_sb[:, kt * 256:(kt + 1) * 256],
                                in_=wkr2[kt * 128:(kt + 1) * 128, :])
        nc.sync.dma_start(out=mult_sb[:], in_=mult[:, :])
        for lt in range(NLT):
            nc.sync.dma_start(out=wq_sb[:, lt * 512:(lt + 1) * 512],
                              in_=wq[lt * 128:(lt + 1) * 128, :])
            nc.gpsimd.dma_start(out=wuk2_sb[:, lt * 256:(lt + 1) * 256],
                                in_=wuk2[lt * 128:(lt + 1) * 128, :])
            nc.sync.dma_start(out=wuv_sb[:, lt * 512:(lt + 1) * 512],
                              in_=wuv[lt * 128:(lt + 1) * 128, :])
        nc.gpsimd.dma_start(out=masks_sb[:], in_=masks[:, :])
        for lt in range(NLT):
            nc.gpsimd.dma_start(out=wout_sb[:, lt * D_MODEL:(lt + 1) * D_MODEL],
                                in_=wout[lt * 128:(lt + 1) * 128, :])

        for _rep in range(reps):
            pending = []       # out-projection thunks from the previous q-block

            def emit_pending(budget):
                for _ in range(min(budget, len(pending))):
                    pending.pop(0)()

            def dma_xts_half(sn, hx):
                xt_t = pX.tile([128, 8 * 512], bf16, name=f"xts{sn}_{hx}", tag="xts")
                for k8 in range(8):
                    kt = hx * 8 + k8
                    nc.sync.dma_start(
                        out=xt_t[:, k8 * 512:(k8 + 1) * 512],
                        in_=xT[kt * 128:(kt + 1) * 128, sn * 512:(sn + 1) * 512])
                return xt_t

            prefetched = dma_xts_half(0, 0)
            for seg in range(NSN):
                sn = seg
                # ================= A(sn): latents + projections =================
                xth = [prefetched, dma_xts_half(sn, 1)]

                def xts_t(kt):
                    return xth[kt // 8][:, (kt % 8) * 512:(kt % 8 + 1) * 512]

                cps_all = {}
                for cname, wd_sb in (("q", wdq_sb), ("kv", wdkv_sb)):
                    for ltp in range(2):
                        cp2 = psBig.tile([128, 1024], f32, tag="mm",
                                         name=f"cp2{cname}{sn}{ltp}")
                        for half in range(2):
                            lt = 2 * ltp + half
                            for kt in range(NKT):
                                nc.tensor.matmul(
                                    cp2[:, half * 512:(half + 1) * 512],
                                    wd_sb[:, kt * D_KV + lt * 128: kt * D_KV + (lt + 1) * 128],
                                    xts_t(kt),
                                    start=(kt == 0), stop=(kt == NKT - 1))
                            cps = pCp.tile([128, 512], bf16, tag="cpre",
                                           name=f"cpre{cname}{sn}{lt}")
                            nc.scalar.copy(cps[:], cp2[:, half * 512:(half + 1) * 512])
                            cps_all[(cname, lt)] = cps

                # kr chains for this sn (keeps PE busy while stats settle)
                kp2 = psBig.tile([128, 1024], f32, tag="mm", name=f"kr2{sn}")
                for p in range(2):
                    for kt in range(NKT):
                        nc.tensor.matmul(
                            kp2[:, p * 512:(p + 1) * 512],
                            wkr2_sb[:, kt * 256 + p * 128: kt * 256 + (p + 1) * 128],
                            xts_t(kt),
                            start=(kt == 0), stop=(kt == NKT - 1))

                # rmsnorm stats: squares on Act, adds on DVE, one gpsimd
                # partition_all_reduce per latent, then
                # bstd = exp(-0.5*ln(ss/512+eps)) on Act (same table set as
                # the attention exp -> no act-table swaps)
                cn = {}
                for cname in ("q", "kv"):
                    sqs = []
                    for lt in range(NLT):
                        sq = pSq.tile([128, 512], bf16, tag="sq",
                                      name=f"sq{cname}{sn}{lt}")
                        nc.scalar.activation(sq[:], cps_all[(cname, lt)][:],
                                             AF.Square)
                        sqs.append(sq)
                    s01 = pSq.tile([128, 512], f32, tag="sqs", bufs=3, name=f"s01{cname}{sn}")
                    s23 = pSq.tile([128, 512], f32, tag="sqs", bufs=3, name=f"s23{cname}{sn}")
                    stot = pSq.tile([128, 512], f32, tag="sqs", bufs=3, name=f"stot{cname}{sn}")
                    nc.vector.tensor_tensor(s01[:], sqs[0][:], sqs[1][:], OP.add)
                    nc.vector.tensor_tensor(s23[:], sqs[2][:], sqs[3][:], OP.add)
                    nc.vector.tensor_tensor(stot[:], s01[:], s23[:], OP.add)
                    sall = pBc.tile([128, 512], f32, tag="tmp", name=f"sall{cname}{sn}")
                    nc.gpsimd.partition_all_reduce(sall[:], stot[:], 128,
                                                   bass_isa.ReduceOp.add)
                    lnv = pBc.tile([128, 512], f32, tag="tmp", name=f"lnv{cname}{sn}")
                    nc.scalar.activation(lnv[:], sall[:], AF.Ln,
                                         bias=epsb[:], scale=1.0 / D_KV)
                    bstd = pBc.tile([128, 512], f32, tag="tmp",
                                    name=f"bstd{cname}{sn}")
                    nc.scalar.activation(bstd[:], lnv[:], AF.Exp, scale=-0.5)
                    cfull = pCn.tile([128, NLT * 512], bf16, tag="cn",
                                     name=f"cn{cname}{sn}")
                    for lt in range(NLT):
                        nc.vector.tensor_tensor(
                            cfull[:, lt * 512:(lt + 1) * 512],
                            cps_all[(cname, lt)][:], bstd[:], OP.mult)
                    cn[cname] = cfull

                if sn + 1 < NSN:
                    prefetched = dma_xts_half(sn + 1, 0)

                # kr rope-multiply into kt_sb
                for p in range(2):
                    h0, h1 = 2 * p, 2 * p + 1
                    kp = kp2[:, p * 512:(p + 1) * 512]
                    m0 = mult_sb[:, 0 * SEQ + sn * 512: 0 * SEQ + (sn + 1) * 512]
                    m1 = mult_sb[:, 1 * SEQ + sn * 512: 1 * SEQ + (sn + 1) * 512]
                    k0 = kt_sb[:, h0 * SEQ + sn * 512: h0 * SEQ + (sn + 1) * 512]
                    k1 = kt_sb[:, h1 * SEQ + sn * 512: h1 * SEQ + (sn + 1) * 512]
                    nc.vector.tensor_tensor(k0[64:128, :], kp[64:128, :], m0[64:128, :], OP.mult)
                    nc.vector.tensor_tensor(k1[0:64, :], kp[0:64, :], m1[0:64, :], OP.mult)

                def cnq(lt):
                    return cn["q"][:, lt * 512:(lt + 1) * 512]

                def cnkv(lt):
                    return cn["kv"][:, lt * 512:(lt + 1) * 512]

                # q projections (pairs of heads share one PSUM pair-tile)
                for hp in range(2):
                    qp2 = psBig.tile([128, 1024], f32, tag="mm", name=f"qp2{sn}{hp}")
                    for half in range(2):
                        hl = 2 * hp + half
                        for lt in range(NLT):
                            nc.tensor.matmul(
                                qp2[:, half * 512:(half + 1) * 512],
                                wq_sb[:, lt * 512 + hl * 128: lt * 512 + (hl + 1) * 128],
                                cnq(lt),
                                start=(lt == 0), stop=(lt == NLT - 1))
                    for half in range(2):
                        hl = 2 * hp + half
                        nc.vector.tensor_tensor(
                            qt_sb[:, hl * SEQ + sn * 512: hl * SEQ + (sn + 1) * 512],
                            qp2[:, half * 512:(half + 1) * 512],
                            mult_sb[:, (hl % 2) * SEQ + sn * 512: (hl % 2) * SEQ + (sn + 1) * 512],
                            OP.mult)
                # k-base projections
                up2 = psBig.tile([128, 1024], f32, tag="mm", name=f"up2{sn}")
                for p in range(2):
                    for lt in range(NLT):
                        nc.tensor.matmul(
                            up2[:, p * 512:(p + 1) * 512],
                            wuk2_sb[:, lt * 256 + p * 128: lt * 256 + (p + 1) * 128],
                            cnkv(lt),
                            start=(lt == 0), stop=(lt == NLT - 1))
                for p in range(2):
                    h0, h1 = 2 * p, 2 * p + 1
                    up = up2[:, p * 512:(p + 1) * 512]
                    m0 = mult_sb[:, 0 * SEQ + sn * 512: 0 * SEQ + (sn + 1) * 512]
                    m1 = mult_sb[:, 1 * SEQ + sn * 512: 1 * SEQ + (sn + 1) * 512]
                    k0 = kt_sb[:, h0 * SEQ + sn * 512: h0 * SEQ + (sn + 1) * 512]
                    k1 = kt_sb[:, h1 * SEQ + sn * 512: h1 * SEQ + (sn + 1) * 512]
                    nc.vector.tensor_tensor(k0[0:64, :], up[0:64, :], m0[0:64, :], OP.mult)
                    nc.vector.tensor_tensor(k1[64:128, :], up[64:128, :], m1[64:128, :], OP.mult)
                # v projections: [seq part, (h d) free], pairs of 128-row s-tiles
                for stp in range(2):
                    vp2 = psBig.tile([128, 1024], f32, tag="mm", name=f"vp2{sn}{stp}")
                    for half in range(2):
                        st = 2 * stp + half
                        for lt in range(NLT):
                            nc.tensor.matmul(
                                vp2[:, half * 512:(half + 1) * 512],
                                cnkv(lt)[:, st * 128:(st + 1) * 128],
                                wuv_sb[:, lt * 512:(lt + 1) * 512],
                                start=(lt == 0), stop=(lt == NLT - 1))
                    s_tile0 = sn * 4 + 2 * stp
                    nc.scalar.copy(
                        v_sb[:, s_tile0 * 512:(s_tile0 + 2) * 512], vp2[:])

                # ================= B(qb=seg): attention q-block =================
                qb = seg
                npair = (qb + 1) * 2
                av_sb = {}
                # budget out-projection filler: spread previous block's 16 chains
                nslots = npair * 2
                per_slot = -(-len(pending) // nslots) if pending else 0
                for hp in range(2):
                    heads = (2 * hp, 2 * hp + 1)
                    av = {}
                    for h in heads:
                        av[h] = psAv.tile([128, 512], f32, tag="av",
                                          name=f"av{qb}{h}")
                    # per-partition softmax partial sums for both heads of the
                    # pair, bf16 (2x DVE), reduced by ONE gpsimd call
                    za2 = pZa.tile([128, 1024], bf16, tag="za",
                                   name=f"za2{qb}{hp}")

                    def zsl(h, lo, hi):
                        zoff = (h % 2) * 512
                        return za2[:, zoff + lo: zoff + hi]
                    es2s = {}
                    for jp in range(npair):
                        jt0, jt1 = 2 * jp, 2 * jp + 1
                        kd0, kd1 = jt0 - qb * 4, jt1 - qb * 4
                        c0e = max(kd0, 0) * 128
                        c0o = max(kd1, 0) * 128
                        for h in heads:
                            sp2 = psBig.tile([128, 1024], f32, tag="mm",
                                             name=f"sp{qb}{h}{jp}")
                            nc.tensor.matmul(
                                sp2[:, c0e:512],
                                kt_sb[:, h * SEQ + jt0 * 128: h * SEQ + (jt0 + 1) * 128],
                                qt_sb[:, h * SEQ + qb * 512 + c0e: h * SEQ + (qb + 1) * 512],
                                start=True, stop=True)
                            # diagonal pairs write the full second half so the
                            # paired exp never reads unwritten PSUM
                            c0w = 0 if kd1 >= 0 else c0o
                            nc.tensor.matmul(
                                sp2[:, 512 + c0w:1024],
                                kt_sb[:, h * SEQ + jt1 * 128: h * SEQ + (jt1 + 1) * 128],
                                qt_sb[:, h * SEQ + qb * 512 + c0w: h * SEQ + (qb + 1) * 512],
                                start=True, stop=True)
                            es2s[h] = (sp2, jp)
                        emit_pending(per_slot)
                        for h in heads:
                            sp2, _ = es2s[h]
                            es2 = pEs.tile([128, 1024], bf16, tag="es",
                                           name=f"es{qb}{h}{jp}")
                            nc.scalar.activation(es2[:, c0e:], sp2[:, c0e:],
                                                 AF.Exp, scale=SCALE)
                            if kd0 >= 0:
                                if c0e > 0:
                                    nc.vector.memset(es2[:, 0:c0e], 0.0)
                                nc.vector.tensor_tensor(
                                    es2[:, c0e:512], es2[:, c0e:512],
                                    mask_sl(kd0, c0e, 512), OP.mult)
                                nc.vector.tensor_tensor(
                                    es2[:, 512:1024], es2[:, 512:1024],
                                    mask_sl(kd1, 0, 512), OP.mult)
                            if jp == 0:
                                nc.vector.tensor_copy(zsl(h, 0, 512), es2[:, 0:512])
                            else:
                                nc.vector.tensor_tensor(zsl(h, c0e, 512), zsl(h, c0e, 512),
                                                        es2[:, c0e:512], OP.add)
                            nc.vector.tensor_tensor(zsl(h, c0e, 512), zsl(h, c0e, 512),
                                                    es2[:, 512 + c0e:1024], OP.add)
                            for half, jt in ((0, jt0), (1, jt1)):
                                vsl = v_sb[:, (jt * H_LOC + h) * 128:
                                           (jt * H_LOC + h + 1) * 128]
                                nc.tensor.matmul(
                                    av[h][:],
                                    vsl,
                                    es2[:, half * 512:(half + 1) * 512],
                                    start=(jp == 0 and half == 0),
                                    stop=(jp == npair - 1 and half == 1))
                    zall2 = pBc.tile([128, 1024], bf16, tag="z2", bufs=2,
                                     name=f"zall2{qb}{hp}")
                    nc.gpsimd.partition_all_reduce(zall2[:], za2[:], 128,
                                                   bass_isa.ReduceOp.add)
                    rzb2 = pBc.tile([128, 1024], bf16, tag="z2", bufs=2,
                                    name=f"rzb2{qb}{hp}")
                    with nc.allow_low_precision(reason="1/z at bf16 is within budget"):
                        nc.vector.reciprocal(rzb2[:], zall2[:])
                    for h in heads:
                        zoff = (h % 2) * 512
                        avs = pAv.tile([128, 512], bf16, tag="avs", name=f"avs{qb}{h}")
                        nc.vector.tensor_tensor(avs[:], av[h][:],
                                                rzb2[:, zoff:zoff + 512], OP.mult)
                        av_sb[h] = avs

                # flush any remaining previous-block out-projection chains
                emit_pending(len(pending))

                # queue this block's out-projection (consumed during block qb+1)
                def mk_thunk(qb, av_tiles, st, ncol, parity):
                    def thunk():
                        yp = psYp.tile([128, 512], f32, tag="yp",
                                       name=f"yp{qb}{st}{ncol}")
                        for f in range(H_LOC):
                            nc.tensor.matmul(
                                yp[:],
                                av_tiles[f][:, st * 128:(st + 1) * 128],
                                wout_sb[:, f * D_MODEL + ncol * 512: f * D_MODEL + (ncol + 1) * 512],
                                start=(f == 0), stop=(f == H_LOC - 1))
                        ys = pYs.tile([128, 512], bf16, tag="ysb",
                                      name=f"ys{qb}{st}{ncol}")
                        nc.vector.tensor_copy(ys[:], yp[:])
                        row0 = qb * 512 + st * 128
                        nc.sync.dma_start(out=y[row0:row0 + 128, ncol * 512:(ncol + 1) * 512],
                                     in_=ys[:])
                    return thunk

                for st in range(4):
                    for ncol in range(4):
                        pending.append(mk_thunk(qb, av_sb, st, ncol, (st + ncol) % 2))

            # final block's out-projection
            emit_pending(len(pending))

    nc.finalize()
    _BUILD_CACHE[reps] = nc
    return nc


def _rope_mult():
    half = D_ROPE // 2
    theta = 1.0 / (ROPE_BASE ** (np.arange(0, D_HEAD, 2, dtype=np.float32) / D_HEAD))
    idx = np.arange(SEQ, dtype=np.float32)[:, None] * theta[None, :]
    r = np.tile(np.cos(idx[:, :half]), (1, 2)) + np.tile(np.sin(idx[:, :half]), (1, 2))
    return np.ascontiguousarray(r.T).astype(np.float32)  # [64, SEQ]


def make_inputs(x, W_dq, W_uq, W_dkv, W_uk, W_uv, W_qr, W_kr, g_q, g_kv, W_out, b_out):
    rT = _rope_mult()
    mult = np.empty((128, 2 * SEQ), np.float32)
    mult[0:64, 0:SEQ] = 1.0
    mult[64:128, 0:SEQ] = rT
    mult[0:64, SEQ:] = rT
    mult[64:128, SEQ:] = 1.0
    mult = mult.astype(BF16)

    jl = np.arange(128)[:, None]
    uu = np.arange(896)[None, :]
    masks = (uu >= 384 + jl).astype(np.float32).astype(BF16)

    gq = g_q.astype(np.float32)[:, None]
    gkv = g_kv.astype(np.float32)[:, None]
    Wuq_g = W_uq * gq
    Wqr_g = W_qr * gq
    Wuk_g = W_uk * gkv
    Wuv_g = W_uv * gkv

    in_maps = []
    for core in range(N_CORES):
        b = core // 4
        g = core % 4
        heads = [4 * g + i for i in range(H_LOC)]

        xb = np.ascontiguousarray(x[b].T).astype(BF16)  # [d_model, seq]

        wq_pack = np.empty((D_KV, H_LOC * 128), np.float32)
        for hl, h in enumerate(heads):
            a = Wuq_g[:, h * 64:(h + 1) * 64]
            r = Wqr_g[:, h * 64:(h + 1) * 64]
            blk = np.concatenate([a, r], axis=1) if hl % 2 == 0 else np.concatenate([r, a], axis=1)
            wq_pack[:, hl * 128:(hl + 1) * 128] = blk

        wuk2 = np.empty((D_KV, 256), np.float32)
        wkr2 = np.empty((D_MODEL, 256), np.float32)
        for p in range(2):
            h0, h1 = heads[2 * p], heads[2 * p + 1]
            wuk2[:, p * 128: p * 128 + 64] = Wuk_g[:, h0 * 64:(h0 + 1) * 64]
            wuk2[:, p * 128 + 64: p * 128 + 128] = Wuk_g[:, h1 * 64:(h1 + 1) * 64]
            wkr2[:, p * 128: p * 128 + 64] = W_kr[:, h1 * 64:(h1 + 1) * 64]
            wkr2[:, p * 128 + 64: p * 128 + 128] = W_kr[:, h0 * 64:(h0 + 1) * 64]

        wuv_pack = np.concatenate(
            [Wuv_g[:, h * 128:(h + 1) * 128] for h in heads], axis=1)
        wout_pack = np.concatenate(
            [W_out[h * 128:(h + 1) * 128, :] for h in heads], axis=0)

        in_maps.append({
            "xT": xb,
            "wdq": W_dq.astype(BF16),
            "wdkv": W_dkv.astype(BF16),
            "wq": wq_pack.astype(BF16),
            "wuk2": wuk2.astype(BF16),
            "wkr2": wkr2.astype(BF16),
            "wuv": wuv_pack.astype(BF16),
            "wout": wout_pack.astype(BF16),
            "mult": mult,
            "masks": masks,
        })
    return in_maps


def kernel(**inputs):
    inputs = {k: np.asarray(v) for k, v in inputs.items()}
    in_maps = make_inputs(
        inputs["x"], inputs["W_dq"], inputs["W_uq"], inputs["W_dkv"],
        inputs["W_uk"], inputs["W_uv"], inputs["W_qr"], inputs["W_kr"],
        inputs["g_q"], inputs["g_kv"], inputs["W_out"], inputs["b_out"])

    nc = build_program(reps=1)
    from concourse.bass_utils import run_bass_kernel_spmd
    res = run_bass_kernel_spmd(nc, in_maps, list(range(N_CORES)))

    b_out = inputs["b_out"].astype(np.float32)
    out = np.zeros((BATCH, SEQ, D_MODEL), np.float32)
    for core in range(N_CORES):
        out[core // 4] += res.results[core]["y"].astype(np.float32)
    out += b_out[None, None, :]
    return out

